# revision 23
# baseline (speedup 1.0000x reference)
"""Trainium2 Bass kernel for nn_MultiHeadedAttention_19713899889501.

Strategy: pure data-parallel over batch (B=8 -> 8 NeuronCores), no
collectives.  Per core, one batch element:

  qagg[t] = sum_{j<5} q[t+j]                  (unweighted window sum)
  kagg[t] = sum_j softmax_j(<k[t+4],k[t+j]>/sqrt(D)) k[t+j]
  vlin    = v[4:] @ W0 + b0
  out     = MHA(qagg, kagg, vlin) @ Wout + bout

Device layout is fully transposed (d on partitions, seq on free dim):
  - qaggT via DVE shift-add tree on host-pre-transposed qT
  - local scores via DVE products + PE ones-column (M=1) reduces; the
    softmax normalization of the local weights is folded into a kagg
    pre-scale (1/wsum); kagg runs fully in bf16 (DVE 2x/4x modes)
  - QK^T head-paired: two K=64 matmuls on rows 0-63 / 64-127
  - exp on ScalarE over (128, 1024) two-bank PSUM reads, with a global
    2^-M weight down-scale folded into the exp bias (softmax-invariant);
    the s-loop is software-pipelined (QK of st+1 issues before att@V of
    st)
  - att@V in the transposed orientation: out[t-block 128, dk 64+1] with
    lhsT = exp-tile slice and rhs = [v_head 64 cols | ones col], so each
    s-tile costs 8x65 PE columns instead of 2x512, and the softmax
    denominator lands in psum column 64 of each 65-block
  - normalization via per-partition-scalar tensor_scalar (x * 1/denom),
    written as bf16 [128t, 128d] tiles; PE transpose (identity matmul)
    flips them back to X^T layout; GpSimd copies psum->sbuf
  - kagg for chunks 1..3 is emitted c-major and interleaved with the
    first SDPA windows' normalization work on DVE
  - output linear with bias via K=1 ones-row matmul, after the last pair

The t/s grid is padded 2044 -> 2048; padded key positions are nulled by
zeroing their vlin rows (including the ones columns, so they drop out of
both the att@V numerator and the denominator); padded t rows are simply
not written back.
"""
import sys

if "/opt/trn_rl_repo" not in sys.path:
    sys.path.insert(0, "/opt/trn_rl_repo")

import numpy as np
import ml_dtypes

import concourse.bass as bass
import concourse.tile as tile
import concourse.mybir as mybir
from concourse import bacc
from concourse.bass_utils import run_bass_kernel_spmd

BF16 = ml_dtypes.bfloat16
FP16 = np.float16
F32 = mybir.dt.float32
FP = mybir.dt.float16
BF = mybir.dt.bfloat16
AF = mybir.ActivationFunctionType
ALU = mybir.AluOpType

B, S, D, H, L = 8, 2048, 512, 8, 5
DK = D // H           # 64
SP = S - L + 1        # 2044 true output positions
SPP = 2048            # padded t/s grid
SQ = S + 8            # padded qT/kT width (2056)
NCH = 4               # d chunks of 128
NT = 4                # t quarters of 512 (prefix)
NS = 16               # s tiles of 128
VW = DK + 1           # 65: v-cols + ones col per head
MEXP = 6.0            # global 2^-M weight scale (softmax-invariant)
SCHRAUD_ST = ()           # s-tiles whose exp runs on DVE via the fp16
                          # Schraudolph bit-trick instead of ScalarE
SCH_C = 0.0433            # minimax correction for 2^f ~ 1+f

N_CORES = 8

_PROGRAM = None


def _build_core_program():
    nc = bacc.Bacc("TRN2", target_bir_lowering=False, debug=False)

    qT = nc.dram_tensor("qT", [D, SQ], FP, kind="ExternalInput").ap()
    kT = nc.dram_tensor("kT", [D, SQ], FP, kind="ExternalInput").ap()
    vT = nc.dram_tensor("vT", [D, SPP], FP, kind="ExternalInput").ap()
    W0 = nc.dram_tensor("W0", [D, D], FP, kind="ExternalInput").ap()
    Wout = nc.dram_tensor("Wout", [D, D], FP, kind="ExternalInput").ap()
    b0 = nc.dram_tensor("b0", [1, D], FP, kind="ExternalInput").ap()
    bout = nc.dram_tensor("bout", [1, D], FP, kind="ExternalInput").ap()
    zpad = nc.dram_tensor("zpad", [4, H * VW], FP, kind="ExternalInput").ap()
    ident = nc.dram_tensor("ident", [128, 128], FP, kind="ExternalInput").ap()
    out = nc.dram_tensor("out", [SP, D], F32, kind="ExternalOutput").ap()

    with tile.TileContext(nc) as tc:
        _build(tc, qT, kT, vT, W0, Wout, b0, bout, zpad, ident, out)
    nc.compile()
    return nc


def _build(tc, qT, kT, vT, W0, Wout, b0, bout, zpad, ident, out):
    nc = tc.nc
    from contextlib import ExitStack

    inv_sqrt_d = float(1.0 / np.sqrt(np.float32(D)))
    inv_sqrt_dk = float(1.0 / np.sqrt(np.float32(DK)))
    exp_bias = float(-MEXP * np.log(2.0))
    sch_a = float(1024.0 * np.log2(np.e) / np.sqrt(np.float32(DK)))
    sch_b = float(1024.0 * (15.0 - MEXP - SCH_C))
    I16 = mybir.dt.int16

    with ExitStack() as ctx:
        ctx.enter_context(nc.allow_low_precision(
            reason="bf16 window sums/weights; validated vs fp32 reference"))
        pers = ctx.enter_context(tc.tile_pool(name="pers", bufs=1))

        # ---------------- constants / weights ----------------
        ones_row = pers.tile([1, 128], FP, tag="ones_row")
        nc.vector.memset(ones_row[:], 1.0)
        ones_col = pers.tile([128, 1], FP, tag="ones_col")
        nc.vector.memset(ones_col[:], 1.0)
        ones_col_last = pers.tile([128, 1], FP, tag="ones_col_last")
        nc.vector.memset(ones_col_last[:], 1.0)

        kTb = []
        _qs = [nc.sync, nc.gpsimd, nc.scalar, nc.sync]
        for c in range(NCH):
            t = pers.tile([128, SQ], FP, tag=f"kTb{c}")
            _qs[c].dma_start(t[:], kT[c * 128:(c + 1) * 128, :])
            kTb.append(t)
        b0_sb = pers.tile([1, D], FP, tag="b0")
        nc.sync.dma_start(b0_sb[:], b0[:])
        ident_sb = pers.tile([128, 128], FP, tag="ident")
        nc.sync.dma_start(ident_sb[:], ident[:])
        ebias = pers.tile([128, 1], F32, tag="ebias")
        nc.vector.memset(ebias[:], exp_bias)

        W0b = []
        for c in range(NCH):
            t = pers.tile([128, D], FP, tag=f"W0b{c}")
            nc.sync.dma_start(t[:], W0[c * 128:(c + 1) * 128, :])
            W0b.append(t)
        Woutb = []
        for c in range(NCH):
            t = pers.tile([128, D], FP, tag=f"Woutb{c}")
            nc.sync.dma_start(t[:], Wout[c * 128:(c + 1) * 128, :])
            Woutb.append(t)
        bout_sb = pers.tile([1, D], FP, tag="bout")
        nc.sync.dma_start(bout_sb[:], bout[:])

        # persistent results of the prefix
        qaggT = []
        kaggT = []
        XhatT = []
        for c in range(NCH):
            qaggT.append(pers.tile([128, SPP], FP, tag=f"qaggT{c}", name=f"qaggT{c}"))
            kaggT.append(pers.tile([128, SPP], FP, tag=f"kaggT{c}", name=f"kaggT{c}"))
            XhatT.append(pers.tile([128, SPP], FP, tag=f"XhatT{c}", name=f"XhatT{c}"))
        # per head h: cols [65h : 65h+64] = v head cols, col 65h+64 = ones
        vlin_sb = []
        for st in range(NS):
            vlin_sb.append(pers.tile([128, H * VW], FP, tag=f"vlin{st}", name=f"vlin{st}"))
        # local-softmax broadcast tiles, persistent across sweeps
        ebs = [[pers.tile([128, 512], FP, tag=f"eb{t4}_{j}", name=f"eb{t4}_{j}") for j in range(L)]
               for t4 in range(NT)]

        # SDPA-era SBUF pools open FIRST so their space does not reuse
        # prefix-pool space (which would chain SDPA startup to prefix tails).
        pap = ctx.enter_context(tc.tile_pool(name="pap", bufs=3))
        schp = ctx.enter_context(tc.tile_pool(name="schp", bufs=2))
        xnp = ctx.enter_context(tc.tile_pool(name="xnp", bufs=3))
        osbp = ctx.enter_context(tc.tile_pool(name="osbp", bufs=2))

        # chunk-0 qagg tree first: its qT DMA precedes the vT transfers,
        # and it gates the first SDPA head pair.
        treep = ctx.enter_context(tc.tile_pool(name="treep", bufs=1))

        def qagg_tree(c):
            x = treep.tile([128, SQ], FP, tag="qT_in")
            nc.gpsimd.dma_start(x[:], qT[c * 128:(c + 1) * 128, :])
            s1 = treep.tile([128, 2052], FP, tag="tree1")
            nc.vector.tensor_add(s1[:], x[:, 0:2052], x[:, 1:2053])
            s2 = treep.tile([128, SPP], FP, tag="tree2")
            nc.vector.tensor_add(s2[:], s1[:, 0:SPP], s1[:, 2:2 + SPP])
            nc.vector.tensor_add(qaggT[c][:], s2[:], x[:, 4:4 + SPP])

        qagg_tree(0)

        # ======== prefix: vlin groups interleaved with local scores ========
        wrowp = ctx.enter_context(tc.tile_pool(name="wrowp", bufs=2))
        kwp = ctx.enter_context(tc.tile_pool(name="kwp", bufs=3))
        if True:
            pre = ExitStack()
            vtp = pre.enter_context(tc.tile_pool(name="vtp", bufs=1))
            pre_ps = pre.enter_context(
                tc.tile_pool(name="pre_ps", bufs=2, space="PSUM"))
            prodp = pre.enter_context(tc.tile_pool(name="prodp", bufs=2))
            e4p = pre.enter_context(tc.tile_pool(name="e4p", bufs=1))
            scr_ps = pre.enter_context(
                tc.tile_pool(name="scr_ps", bufs=1, space="PSUM"))

            vtbs = [[None] * 4 for _ in range(NCH)]

            def vlin_group(g):
                # vT DMAs for this group, then 4 vlin st-blocks on PE/Pool
                for c in range(NCH):
                    vtb = vtp.tile([128, 512], FP, tag=f"vTb{c}_{g}",
                                   name=f"vTb{c}_{g}")
                    nc.sync.dma_start(
                        vtb[:], vT[c * 128:(c + 1) * 128,
                                   g * 512:(g + 1) * 512])
                    vtbs[c][g] = vtb
                for st in range(4 * g, 4 * g + 4):
                    o = (st % 4) * 128
                    ps = pre_ps.tile([128, 512], F32, tag="vlin_ps")
                    for c in range(NCH):
                        nc.tensor.matmul(
                            ps[:], vtbs[c][g][:, o:o + 128], W0b[c][:],
                            start=(c == 0), stop=False,
                        )
                    nc.tensor.matmul(ps[:], ones_row[:], b0_sb[:],
                                     start=False, stop=True)
                    vre = vlin_sb[st].rearrange("p (h u) -> p h u", u=VW)
                    psr = ps.rearrange("p (h u) -> p h u", u=64)
                    nc.scalar.activation(vre[:, :, 0:DK], psr[:], AF.Copy)
                    nc.gpsimd.memset(vre[:, :, DK:VW], 1.0)
                    if st == NS - 1:
                        # zero padded key rows (engine ops can't address base
                        # 124; DMA is address-based and can). Covers the ones
                        # cols too, so padded s drop out of numerator AND
                        # denominator.
                        nc.sync.dma_start(vlin_sb[st][124:128, :], zpad[:])
                        nc.sync.dma_start(ones_col_last[124:128, :],
                                          zpad[0:4, 0:1])

            rrow_keep = []

            def products_scores(t4):
                scr4 = scr_ps.tile([1, 5 * 512], F32, tag="scr4")
                for j in range(L):
                    for c in range(NCH):
                        p = prodp.tile([128, 512], FP, tag="prod")
                        nc.vector.tensor_mul(
                            p[:],
                            kTb[c][:, t4 * 512 + 4:t4 * 512 + 4 + 512],
                            kTb[c][:, t4 * 512 + j:t4 * 512 + j + 512])
                        nc.tensor.matmul(
                            scr4[:, j * 512:(j + 1) * 512],
                            ones_col[:], p[:],
                            start=(c == 0), stop=(c == NCH - 1),
                        )
                e4 = e4p.tile([1, 5 * 512], F32, tag="e4")
                nc.scalar.activation(e4[:], scr4[:], AF.Exp, scale=inv_sqrt_d)
                return e4

            def wsum_recip_bcast(t4, e4):
                w1 = wrowp.tile([1, 512], F32, tag="w1")
                nc.vector.tensor_add(w1[:], e4[:, 0:512], e4[:, 512:1024])
                w2 = wrowp.tile([1, 512], F32, tag="w2")
                nc.vector.tensor_add(w2[:], e4[:, 1024:1536], e4[:, 1536:2048])
                w3 = wrowp.tile([1, 512], F32, tag="w3")
                nc.vector.tensor_add(w3[:], w1[:], w2[:])
                wsum = wrowp.tile([1, 512], F32, tag="wsum")
                nc.vector.tensor_add(wsum[:], w3[:], e4[:, 2048:2560])
                rrow = pers.tile([1, 512], F32, tag=f"rrow{t4}", name=f"rrow{t4}")
                nc.vector.reciprocal(rrow[:], wsum[:])
                rrow_keep.append(rrow)
                for j in range(L):
                    wj = wrowp.tile([1, 512], FP, tag="wj")
                    nc.vector.tensor_mul(
                        wj[:], e4[:, j * 512:(j + 1) * 512], rrow[:])
                    nc.gpsimd.partition_broadcast(ebs[t4][j][:], wj[:])

            def kagg_quarter(c, t4):
                sl = slice(t4 * 512, (t4 + 1) * 512)
                acc = kwp.tile([128, 512], FP, tag="kacc")
                nc.vector.tensor_mul(
                    acc[:], kTb[c][:, t4 * 512:t4 * 512 + 512], ebs[t4][0][:])
                for j in range(1, L):
                    term = kwp.tile([128, 512], FP, tag="kterm")
                    nc.vector.tensor_mul(
                        term[:], kTb[c][:, t4 * 512 + j:t4 * 512 + j + 512],
                        ebs[t4][j][:])
                    dst = kaggT[c][:, sl] if j == L - 1 else \
                        kwp.tile([128, 512], FP, tag="kacc")
                    nc.vector.tensor_add(dst[:], acc[:], term[:])
                    acc = dst

            # pipelined emission: the scr4 chain runs ahead of the vlin
            # groups on PE (SDPA start is gated by kagg, not vlin), while
            # wsum/bcast/kagg0 trail one quarter behind on DVE/Pool.
            e4s = [None] * NT
            e4s[0] = products_scores(0)
            e4s[1] = products_scores(1)
            vlin_group(0)
            wsum_recip_bcast(0, e4s[0])
            kagg_quarter(0, 0)
            e4s[2] = products_scores(2)
            vlin_group(1)
            wsum_recip_bcast(1, e4s[1])
            kagg_quarter(0, 1)
            e4s[3] = products_scores(3)
            vlin_group(2)
            wsum_recip_bcast(2, e4s[2])
            kagg_quarter(0, 2)
            vlin_group(3)
            wsum_recip_bcast(3, e4s[3])
            kagg_quarter(0, 3)

            for _c in range(1, NCH):
                qagg_tree(_c)
            pre.close()  # release prefix PSUM banks for the SDPA pools

            # ================= SDPA main loop (+ sweep-2 kagg) =============
            with ExitStack() as main:
                qk_ps = main.enter_context(
                    tc.tile_pool(name="qk_ps", bufs=2, space="PSUM"))
                x_ps_pool = main.enter_context(
                    tc.tile_pool(name="x_ps", bufs=1, space="PSUM"))
                xt_ps_pool = main.enter_context(
                    tc.tile_pool(name="xt_ps", bufs=1, space="PSUM"))
                o_ps_pool = main.enter_context(
                    tc.tile_pool(name="o_ps", bufs=1, space="PSUM"))
                rxp = main.enter_context(tc.tile_pool(name="rxp", bufs=2))

                # sweep-2 kagg quarters (chunks 1..3, c-major), doled out a
                # few per SDPA window so DVE norm work is never starved.
                sweep2 = [(c, t4) for c in range(1, NCH) for t4 in range(NT)]
                s2i = 0

                def outlin_block(tb):
                    o_ps = o_ps_pool.tile([128, 512], F32, tag="o_ps")
                    for c in range(NCH):
                        nc.tensor.matmul(
                            o_ps[:], XhatT[c][:, tb * 128:(tb + 1) * 128],
                            Woutb[c][:], start=(c == 0), stop=False,
                        )
                    nc.tensor.matmul(o_ps[:], ones_row[:], bout_sb[:],
                                     start=False, stop=True)
                    o_sb = osbp.tile([128, 512], F32, tag="o_sb")
                    nc.vector.tensor_copy(o_sb[:], o_ps[:])
                    rows = 128 if tb < NS - 1 else SP - 128 * (NS - 1)
                    nc.sync.dma_start(out[tb * 128: tb * 128 + rows, :],
                                      o_sb[0:rows, :])

                def attv_block(xps, den, pa, pair, pst):
                    # 8 att@V matmuls into one bank-exact accumulator tile
                    # (cols (4*hx+tb)*64) + 8 K-column denominator matmuls.
                    # One accumulation group per psum bank: only the very
                    # first matmul of the bank carries start=True.
                    oc = ones_col_last if pst == NS - 1 else ones_col
                    for hx in range(2):
                        vsl = slice((2 * pair + hx) * VW,
                                    (2 * pair + hx) * VW + DK)
                        for tb in range(4):
                            b = 4 * hx + tb
                            pasl = pa[:, hx * 512 + tb * 128:
                                      hx * 512 + (tb + 1) * 128]
                            nc.tensor.matmul(
                                xps[:, b * DK:(b + 1) * DK],
                                pasl, vlin_sb[pst][:, vsl],
                                start=(pst == 0 and b == 0),
                                stop=(pst == NS - 1 and b == 7),
                            )
                            nc.tensor.matmul(
                                den[:, b:b + 1], pasl, oc[:],
                                start=(pst == 0 and b == 0),
                                stop=(pst == NS - 1 and b == 7),
                            )

                def emit_qk(c, tcx, st):
                    tsl = slice(tcx * 512, (tcx + 1) * 512)
                    ssl = slice(st * 128, (st + 1) * 128)
                    p_ps = qk_ps.tile([128, 1024], F32, tag="p_ps")
                    nc.tensor.matmul(
                        p_ps[:, 0:512],
                        kaggT[c][0:64, ssl], qaggT[c][0:64, tsl],
                        start=True, stop=True,
                    )
                    nc.tensor.matmul(
                        p_ps[:, 512:1024],
                        kaggT[c][64:128, ssl], qaggT[c][64:128, tsl],
                        start=True, stop=True,
                    )
                    return p_ps

                windows = [(pair, tcx) for pair in range(H // 2)
                           for tcx in range(NT)]
                carry = None
                tail = None   # deferred (pair, tcx, xn) from previous window

                def emit_tail_pe(tpair, ttcx, xn):
                    # transposes + psum->sbuf copy, deferred into the next
                    # window so they never block its first QK/exp on the
                    # in-order PE.
                    xt = xt_ps_pool.tile([128, 512], FP, tag="xt")
                    for tb in range(4):
                        nc.tensor.matmul(
                            xt[:, tb * 128:(tb + 1) * 128], xn[tb][:],
                            ident_sb[:], is_transpose=True,
                            start=(tb == 0), stop=(tb == 3),
                        )
                    nc.vector.tensor_copy(
                        XhatT[tpair][:, ttcx * 512:(ttcx + 1) * 512], xt[:])

                for w, (pair, tcx) in enumerate(windows):
                    c = pair      # chunk c holds heads 2c (rows 0:64), 2c+1
                    xps = x_ps_pool.tile([128, 512], F32, tag="xps")
                    den = x_ps_pool.tile([128, 8], F32, tag="den")
                    if carry is None:
                        carry = [emit_qk(c, tcx, 0), emit_qk(c, tcx, 1),
                                 emit_qk(c, tcx, 2)]
                    pending = None
                    # deferred outlin blocks of the previous pair-3 window,
                    # spread across this window's st iterations
                    outs = []
                    if tail is not None and tail[0] == H // 2 - 1:
                        outs = list(range(4 * tail[1], 4 * tail[1] + 4))
                    # software pipeline: QK(st) is emitted before attV(st-1)
                    # so the PE never stalls behind the exp; the first two
                    # QKs of the NEXT window are pre-issued before this
                    # window's norm tail for the same reason.
                    for st in range(NS):
                        p_ps = (carry[st] if st < 3
                                else emit_qk(c, tcx, st))
                        if st in SCHRAUD_ST:
                            # exp via fp16 bit-trick on DVE: the int16
                            # y = a*score + b IS the fp16 encoding of
                            # 2^(log2e*score/8 - M) up to the (1+f)~2^f
                            # linear-mantissa approximation; max(0) clamps
                            # the below-cutoff tail that would bitcast to
                            # negative weights.
                            yi = schp.tile([128, 1024], I16, tag="pay")
                            nc.vector.tensor_scalar(
                                yi[:], p_ps[:], sch_a, sch_b,
                                ALU.mult, ALU.add)
                            pu = schp.tile([128, 1024], I16, tag="pac")
                            nc.vector.tensor_scalar_max(pu[:], yi[:], 0)
                            pa = pu.bitcast(FP)
                        else:
                            pa = pap.tile([128, 1024], FP, tag="pa")
                            nc.scalar.activation(pa[:], p_ps[:], AF.Exp,
                                                 scale=inv_sqrt_dk,
                                                 bias=ebias[:])
                        if st == 2 and tail is not None:
                            emit_tail_pe(tail[0], tail[1], tail[2])
                            tail = None
                        if st in (3, 6, 9, 12) and outs:
                            outlin_block(outs.pop(0))
                        if pending is not None:
                            attv_block(xps, den, pending[0], pair,
                                       pending[1])
                        pending = (pa, st)
                    if w + 1 < len(windows):
                        npair, ntcx = windows[w + 1]
                        carry = [emit_qk(npair, ntcx, 0),
                                 emit_qk(npair, ntcx, 1)]
                    attv_block(xps, den, pending[0], pair, pending[1])
                    if w + 1 < len(windows):
                        carry.append(emit_qk(npair, ntcx, 2))
                    # normalization: per-partition scalar 1/denom (kept at
                    # the window boundary -- the next window's first attV
                    # overwrites xps and must trail these reads)
                    xre = xps.rearrange("p (b u) -> p b u", u=DK)
                    rx = rxp.tile([128, 8], F32, tag="rx")
                    nc.vector.reciprocal(rx[:], den[:])
                    xn = [xnp.tile([128, 128], FP, tag=f"xn{tb}",
                                   name=f"xn{tb}")
                          for tb in range(4)]
                    for tb in range(4):
                        nc.vector.tensor_scalar(
                            xn[tb][:, 0:64], xre[:, tb, :],
                            rx[:, tb:tb + 1], None, ALU.mult)
                        nc.vector.tensor_scalar(
                            xn[tb][:, 64:128], xre[:, 4 + tb, :],
                            rx[:, 4 + tb:5 + tb], None, ALU.mult)
                    # sweep-2 kagg quarters AFTER the norm ops so they never
                    # delay the next window's attV start on DVE
                    for _ in range(2):
                        if s2i < len(sweep2):
                            kagg_quarter(*sweep2[s2i])
                            s2i += 1
                    tail = (pair, tcx, xn)

                # final window's tail
                emit_tail_pe(tail[0], tail[1], tail[2])
                for tb in range(4 * tail[1], 4 * tail[1] + 4):
                    outlin_block(tb)
                tail = None
                # drain any sweep-2 stragglers (shouldn't happen)
                while s2i < len(sweep2):
                    kagg_quarter(*sweep2[s2i])
                    s2i += 1


def _get_program():
    global _PROGRAM
    if _PROGRAM is None:
        _PROGRAM = _build_core_program()
    return _PROGRAM


def _prep_core_inputs(q, k, v, W0, b0, Wout, bout):
    """Host-side layout prep for one batch element (layout/dtype only)."""
    qTp = np.zeros((D, SQ), FP16)
    qTp[:, 0:S] = np.ascontiguousarray(q.T).astype(FP16)
    kTp = np.zeros((D, SQ), FP16)
    kTp[:, 0:S] = np.ascontiguousarray(k.T).astype(FP16)
    vTp = np.zeros((D, SPP), FP16)
    vTp[:, 0:S - 4] = np.ascontiguousarray(v[4:].T).astype(FP16)
    return {
        "qT": qTp,
        "kT": kTp,
        "vT": vTp,
        "W0": W0.astype(FP16),
        "Wout": Wout.astype(FP16),
        "b0": b0.reshape(1, D).astype(FP16),
        "bout": bout.reshape(1, D).astype(FP16),
        "zpad": np.zeros((4, H * VW), FP16),
        "ident": np.eye(128, dtype=np.float32).astype(FP16),
    }


def kernel(query, key, value, W0, b0, Wout, bout):
    query = np.asarray(query, np.float32)
    key = np.asarray(key, np.float32)
    value = np.asarray(value, np.float32)
    W0 = np.asarray(W0, np.float32)
    b0 = np.asarray(b0, np.float32)
    Wout = np.asarray(Wout, np.float32)
    bout = np.asarray(bout, np.float32)

    nc = _get_program()
    in_maps = [
        _prep_core_inputs(query[b], key[b], value[b], W0, b0, Wout, bout)
        for b in range(B)
    ]
    res = run_bass_kernel_spmd(nc, in_maps, list(range(N_CORES)))
    return np.stack([res.results[b]["out"] for b in range(B)], axis=0)


# revision 26
# speedup vs baseline: 1.0425x; 1.0425x over previous
"""Trainium2 Bass kernel for nn_MultiHeadedAttention_19713899889501.

Strategy: pure data-parallel over batch (B=8 -> 8 NeuronCores), no
collectives.  Per core, one batch element:

  qagg[t] = sum_{j<5} q[t+j]                  (unweighted window sum)
  kagg[t] = sum_j softmax_j(<k[t+4],k[t+j]>/sqrt(D)) k[t+j]
  vlin    = v[4:] @ W0 + b0
  out     = MHA(qagg, kagg, vlin) @ Wout + bout

Device layout is fully transposed (d on partitions, seq on free dim):
  - qaggT via DVE shift-add tree on host-pre-transposed qT
  - local scores via DVE products + PE ones-column (M=1) reduces; the
    softmax normalization of the local weights is folded into a kagg
    pre-scale (1/wsum); kagg runs fully in bf16 (DVE 2x/4x modes)
  - QK^T head-paired: two K=64 matmuls on rows 0-63 / 64-127
  - exp on ScalarE over (128, 1024) two-bank PSUM reads, with a global
    2^-M weight down-scale folded into the exp bias (softmax-invariant);
    the s-loop is software-pipelined (QK of st+1 issues before att@V of
    st)
  - att@V in the transposed orientation: out[t-block 128, dk 64+1] with
    lhsT = exp-tile slice and rhs = [v_head 64 cols | ones col], so each
    s-tile costs 8x65 PE columns instead of 2x512, and the softmax
    denominator lands in psum column 64 of each 65-block
  - normalization via per-partition-scalar tensor_scalar (x * 1/denom),
    written as bf16 [128t, 128d] tiles; PE transpose (identity matmul)
    flips them back to X^T layout; GpSimd copies psum->sbuf
  - kagg for chunks 1..3 is emitted c-major and interleaved with the
    first SDPA windows' normalization work on DVE
  - output linear with bias via K=1 ones-row matmul, after the last pair

The t/s grid is padded 2044 -> 2048; padded key positions are nulled by
zeroing their vlin rows (including the ones columns, so they drop out of
both the att@V numerator and the denominator); padded t rows are simply
not written back.
"""
import sys

if "/opt/trn_rl_repo" not in sys.path:
    sys.path.insert(0, "/opt/trn_rl_repo")

import numpy as np
import ml_dtypes

import concourse.bass as bass
import concourse.tile as tile
import concourse.mybir as mybir
from concourse import bacc
from concourse.bass_utils import run_bass_kernel_spmd

BF16 = ml_dtypes.bfloat16
FP16 = np.float16
F32 = mybir.dt.float32
FP = mybir.dt.float16
BF = mybir.dt.bfloat16
AF = mybir.ActivationFunctionType
ALU = mybir.AluOpType

B, S, D, H, L = 8, 2048, 512, 8, 5
DK = D // H           # 64
SP = S - L + 1        # 2044 true output positions
SPP = 2048            # padded t/s grid
SQ = S + 8            # padded qT/kT width (2056)
NCH = 4               # d chunks of 128
NT = 4                # t quarters of 512 (prefix)
NS = 16               # s tiles of 128
VW = 2 * DK           # 128: v cols + replicated-ones cols per head
MEXP = 6.0            # global 2^-M weight scale (softmax-invariant)
SCHRAUD_ST = ()           # s-tiles whose exp runs on DVE via the fp16
                          # Schraudolph bit-trick instead of ScalarE
SCH_C = 0.0433            # minimax correction for 2^f ~ 1+f

N_CORES = 8

_PROGRAM = None


def _build_core_program():
    nc = bacc.Bacc("TRN2", target_bir_lowering=False, debug=False)

    qT = nc.dram_tensor("qT", [D, SQ], FP, kind="ExternalInput").ap()
    kT = nc.dram_tensor("kT", [D, SQ], FP, kind="ExternalInput").ap()
    vT = nc.dram_tensor("vT", [D, SPP], FP, kind="ExternalInput").ap()
    W0 = nc.dram_tensor("W0", [D, D], FP, kind="ExternalInput").ap()
    Wout = nc.dram_tensor("Wout", [D, D], FP, kind="ExternalInput").ap()
    b0 = nc.dram_tensor("b0", [1, D], FP, kind="ExternalInput").ap()
    bout = nc.dram_tensor("bout", [1, D], FP, kind="ExternalInput").ap()
    zpad = nc.dram_tensor("zpad", [4, H * VW], FP, kind="ExternalInput").ap()
    ident = nc.dram_tensor("ident", [128, 128], FP, kind="ExternalInput").ap()
    out = nc.dram_tensor("out", [SP, D], F32, kind="ExternalOutput").ap()

    with tile.TileContext(nc) as tc:
        _build(tc, qT, kT, vT, W0, Wout, b0, bout, zpad, ident, out)
    nc.compile()
    return nc


def _build(tc, qT, kT, vT, W0, Wout, b0, bout, zpad, ident, out):
    nc = tc.nc
    from contextlib import ExitStack

    inv_sqrt_d = float(1.0 / np.sqrt(np.float32(D)))
    inv_sqrt_dk = float(1.0 / np.sqrt(np.float32(DK)))
    exp_bias = float(-MEXP * np.log(2.0))
    sch_a = float(1024.0 * np.log2(np.e) / np.sqrt(np.float32(DK)))
    sch_b = float(1024.0 * (15.0 - MEXP - SCH_C))
    I16 = mybir.dt.int16

    with ExitStack() as ctx:
        ctx.enter_context(nc.allow_low_precision(
            reason="bf16 window sums/weights; validated vs fp32 reference"))
        pers = ctx.enter_context(tc.tile_pool(name="pers", bufs=1))

        # ---------------- constants / weights ----------------
        ones_row = pers.tile([1, 128], FP, tag="ones_row")
        nc.vector.memset(ones_row[:], 1.0)
        ones_col = pers.tile([128, 1], FP, tag="ones_col")
        nc.vector.memset(ones_col[:], 1.0)

        kTb = []
        _qs = [nc.sync, nc.gpsimd, nc.scalar, nc.sync]
        for c in range(NCH):
            t = pers.tile([128, SQ], FP, tag=f"kTb{c}")
            _qs[c].dma_start(t[:], kT[c * 128:(c + 1) * 128, :])
            kTb.append(t)
        b0_sb = pers.tile([1, D], FP, tag="b0")
        nc.sync.dma_start(b0_sb[:], b0[:])
        ebias = pers.tile([128, 1], F32, tag="ebias")
        nc.vector.memset(ebias[:], exp_bias)

        W0b = []
        for c in range(NCH):
            t = pers.tile([128, D], FP, tag=f"W0b{c}")
            nc.sync.dma_start(t[:], W0[c * 128:(c + 1) * 128, :])
            W0b.append(t)
        Woutb = []
        for c in range(NCH):
            t = pers.tile([128, D], FP, tag=f"Woutb{c}")
            nc.sync.dma_start(t[:], Wout[c * 128:(c + 1) * 128, :])
            Woutb.append(t)
        bout_sb = pers.tile([1, D], FP, tag="bout")
        nc.sync.dma_start(bout_sb[:], bout[:])

        # persistent results of the prefix
        qaggT = []
        kaggT = []
        XhatT = []
        for c in range(NCH):
            qaggT.append(pers.tile([128, SPP], FP, tag=f"qaggT{c}", name=f"qaggT{c}"))
            kaggT.append(pers.tile([128, SPP], FP, tag=f"kaggT{c}", name=f"kaggT{c}"))
            XhatT.append(pers.tile([128, SPP], FP, tag=f"XhatT{c}", name=f"XhatT{c}"))
        # per head h: cols [65h : 65h+64] = v head cols, col 65h+64 = ones
        vlin_sb = []
        for st in range(NS):
            vlin_sb.append(pers.tile([128, H * VW], FP, tag=f"vlin{st}", name=f"vlin{st}"))
        # local-softmax broadcast tiles, persistent across sweeps
        ebs = [[pers.tile([128, 512], FP, tag=f"eb{t4}_{j}", name=f"eb{t4}_{j}") for j in range(L)]
               for t4 in range(NT)]

        # SDPA-era SBUF pools open FIRST so their space does not reuse
        # prefix-pool space (which would chain SDPA startup to prefix tails).
        pap = ctx.enter_context(tc.tile_pool(name="pap", bufs=3))
        osbp = ctx.enter_context(tc.tile_pool(name="osbp", bufs=2))

        # chunk-0 qagg tree first: its qT DMA precedes the vT transfers,
        # and it gates the first SDPA head pair.
        treep = ctx.enter_context(tc.tile_pool(name="treep", bufs=1))

        def qagg_tree(c):
            x = treep.tile([128, SQ], FP, tag="qT_in")
            nc.gpsimd.dma_start(x[:], qT[c * 128:(c + 1) * 128, :])
            s1 = treep.tile([128, 2052], FP, tag="tree1")
            nc.vector.tensor_add(s1[:], x[:, 0:2052], x[:, 1:2053])
            s2 = treep.tile([128, SPP], FP, tag="tree2")
            nc.vector.tensor_add(s2[:], s1[:, 0:SPP], s1[:, 2:2 + SPP])
            nc.vector.tensor_add(qaggT[c][:], s2[:], x[:, 4:4 + SPP])

        qagg_tree(0)

        # ======== prefix: vlin groups interleaved with local scores ========
        wrowp = ctx.enter_context(tc.tile_pool(name="wrowp", bufs=1))
        kwp = ctx.enter_context(tc.tile_pool(name="kwp", bufs=3))
        if True:
            pre = ExitStack()
            vtp = pre.enter_context(tc.tile_pool(name="vtp", bufs=1))
            pre_ps = pre.enter_context(
                tc.tile_pool(name="pre_ps", bufs=2, space="PSUM"))
            prodp = pre.enter_context(tc.tile_pool(name="prodp", bufs=2))
            e4p = pre.enter_context(tc.tile_pool(name="e4p", bufs=1))
            scr_ps = pre.enter_context(
                tc.tile_pool(name="scr_ps", bufs=1, space="PSUM"))

            vtbs = [[None] * 4 for _ in range(NCH)]

            def vlin_group(g):
                # vT DMAs for this group, then 4 vlin st-blocks on PE/Pool
                for c in range(NCH):
                    vtb = vtp.tile([128, 512], FP, tag=f"vTb{c}_{g}",
                                   name=f"vTb{c}_{g}")
                    nc.sync.dma_start(
                        vtb[:], vT[c * 128:(c + 1) * 128,
                                   g * 512:(g + 1) * 512])
                    vtbs[c][g] = vtb
                for st in range(4 * g, 4 * g + 4):
                    o = (st % 4) * 128
                    ps = pre_ps.tile([128, 512], F32, tag="vlin_ps")
                    for c in range(NCH):
                        nc.tensor.matmul(
                            ps[:], vtbs[c][g][:, o:o + 128], W0b[c][:],
                            start=(c == 0), stop=False,
                        )
                    nc.tensor.matmul(ps[:], ones_row[:], b0_sb[:],
                                     start=False, stop=True)
                    vre = vlin_sb[st].rearrange("p (h u) -> p h u", u=VW)
                    psr = ps.rearrange("p (h u) -> p h u", u=64)
                    nc.scalar.activation(vre[:, :, 0:DK], psr[:], AF.Copy)
                    nc.gpsimd.memset(vre[:, :, DK:VW], 1.0)

                    if st == NS - 1:
                        # zero padded key rows (engine ops can't address base
                        # 124; DMA is address-based and can). Covers the ones
                        # cols too, so padded s drop out of numerator AND
                        # denominator.
                        nc.sync.dma_start(vlin_sb[st][124:128, :], zpad[:])

            rrow_keep = []

            def products_scores(t4):
                scr4 = scr_ps.tile([1, 5 * 512], F32, tag="scr4")
                for j in range(L):
                    for c in range(NCH):
                        p = prodp.tile([128, 512], FP, tag="prod")
                        nc.vector.tensor_mul(
                            p[:],
                            kTb[c][:, t4 * 512 + 4:t4 * 512 + 4 + 512],
                            kTb[c][:, t4 * 512 + j:t4 * 512 + j + 512])
                        nc.tensor.matmul(
                            scr4[:, j * 512:(j + 1) * 512],
                            ones_col[:], p[:],
                            start=(c == 0), stop=(c == NCH - 1),
                        )
                e4 = e4p.tile([1, 5 * 512], F32, tag="e4")
                nc.scalar.activation(e4[:], scr4[:], AF.Exp, scale=inv_sqrt_d)
                return e4

            def wsum_recip_bcast(t4, e4):
                w1 = wrowp.tile([1, 512], F32, tag="w1")
                nc.vector.tensor_add(w1[:], e4[:, 0:512], e4[:, 512:1024])
                w2 = wrowp.tile([1, 512], F32, tag="w2")
                nc.vector.tensor_add(w2[:], e4[:, 1024:1536], e4[:, 1536:2048])
                w3 = wrowp.tile([1, 512], F32, tag="w3")
                nc.vector.tensor_add(w3[:], w1[:], w2[:])
                wsum = wrowp.tile([1, 512], F32, tag="wsum")
                nc.vector.tensor_add(wsum[:], w3[:], e4[:, 2048:2560])
                rrow = pers.tile([1, 512], F32, tag=f"rrow{t4}", name=f"rrow{t4}")
                nc.vector.reciprocal(rrow[:], wsum[:])
                rrow_keep.append(rrow)
                for j in range(L):
                    wj = wrowp.tile([1, 512], FP, tag="wj")
                    nc.vector.tensor_mul(
                        wj[:], e4[:, j * 512:(j + 1) * 512], rrow[:])
                    nc.gpsimd.partition_broadcast(ebs[t4][j][:], wj[:])

            def kagg_quarter(c, t4):
                sl = slice(t4 * 512, (t4 + 1) * 512)
                acc = kwp.tile([128, 512], FP, tag="kacc")
                nc.vector.tensor_mul(
                    acc[:], kTb[c][:, t4 * 512:t4 * 512 + 512], ebs[t4][0][:])
                for j in range(1, L):
                    term = kwp.tile([128, 512], FP, tag="kterm")
                    nc.vector.tensor_mul(
                        term[:], kTb[c][:, t4 * 512 + j:t4 * 512 + j + 512],
                        ebs[t4][j][:])
                    dst = kaggT[c][:, sl] if j == L - 1 else \
                        kwp.tile([128, 512], FP, tag="kacc")
                    nc.vector.tensor_add(dst[:], acc[:], term[:])
                    acc = dst

            # pipelined emission: the scr4 chain runs ahead of the vlin
            # groups on PE (SDPA start is gated by kagg, not vlin), while
            # wsum/bcast/kagg0 trail one quarter behind on DVE/Pool.
            e4s = [None] * NT
            e4s[0] = products_scores(0)
            e4s[1] = products_scores(1)
            vlin_group(0)
            wsum_recip_bcast(0, e4s[0])
            kagg_quarter(0, 0)
            e4s[2] = products_scores(2)
            vlin_group(1)
            wsum_recip_bcast(1, e4s[1])
            kagg_quarter(0, 1)
            e4s[3] = products_scores(3)
            vlin_group(2)
            wsum_recip_bcast(2, e4s[2])
            kagg_quarter(0, 2)
            vlin_group(3)
            wsum_recip_bcast(3, e4s[3])
            kagg_quarter(0, 3)

            for _c in range(1, NCH):
                qagg_tree(_c)
            pre.close()  # release prefix PSUM banks for the SDPA pools

            # ================= SDPA main loop (+ sweep-2 kagg) =============
            with ExitStack() as main:
                qk_ps = main.enter_context(
                    tc.tile_pool(name="qk_ps", bufs=2, space="PSUM"))
                x_ps_pool = main.enter_context(
                    tc.tile_pool(name="x_ps", bufs=2, space="PSUM"))
                rxp = main.enter_context(tc.tile_pool(name="rxp", bufs=2))

                # sweep-2 kagg quarters (chunks 1..3, c-major), doled out a
                # few per SDPA window so DVE norm work is never starved.
                sweep2 = [(c, t4) for c in range(1, NCH) for t4 in range(NT)]
                s2i = 0

                def outlin_block(tb):
                    o_ps = x_ps_pool.tile([128, 512], F32, tag="xA")
                    for c in range(NCH):
                        nc.tensor.matmul(
                            o_ps[:], XhatT[c][:, tb * 128:(tb + 1) * 128],
                            Woutb[c][:], start=(c == 0), stop=False,
                        )
                    nc.tensor.matmul(o_ps[:], ones_row[:], bout_sb[:],
                                     start=False, stop=True)
                    o_sb = osbp.tile([128, 512], F32, tag="o_sb")
                    nc.vector.tensor_copy(o_sb[:], o_ps[:])
                    rows = 128 if tb < NS - 1 else SP - 128 * (NS - 1)
                    nc.sync.dma_start(out[tb * 128: tb * 128 + rows, :],
                                      o_sb[0:rows, :])

                def attv_block(xA, xB, pa, pair, pst):
                    # att@V in the [dk, t] orientation: lhsT = vlin head
                    # block (64 v cols | 64 ones cols), rhs = exp tile; the
                    # ones rows replicate the softmax denominator for free.
                    for hx, xps in ((0, xA), (1, xB)):
                        vsl = slice((2 * pair + hx) * VW,
                                    (2 * pair + hx + 1) * VW)
                        nc.tensor.matmul(
                            xps[:], vlin_sb[pst][:, vsl],
                            pa[:, hx * 512:(hx + 1) * 512],
                            start=(pst == 0), stop=(pst == NS - 1),
                        )

                def emit_qk(c, tcx, st):
                    tsl = slice(tcx * 512, (tcx + 1) * 512)
                    ssl = slice(st * 128, (st + 1) * 128)
                    p_ps = qk_ps.tile([128, 1024], F32, tag="p_ps")
                    nc.tensor.matmul(
                        p_ps[:, 0:512],
                        kaggT[c][0:64, ssl], qaggT[c][0:64, tsl],
                        start=True, stop=True,
                    )
                    nc.tensor.matmul(
                        p_ps[:, 512:1024],
                        kaggT[c][64:128, ssl], qaggT[c][64:128, tsl],
                        start=True, stop=True,
                    )
                    return p_ps

                windows = [(pair, tcx) for pair in range(H // 2)
                           for tcx in range(NT)]
                carry = None
                tail = None   # deferred outlin source from previous window

                for w, (pair, tcx) in enumerate(windows):
                    c = pair      # chunk c holds heads 2c (rows 0:64), 2c+1
                    xA = x_ps_pool.tile([128, 512], F32, tag="xA")
                    xB = x_ps_pool.tile([128, 512], F32, tag="xB")
                    if carry is None:
                        carry = [emit_qk(c, tcx, 0), emit_qk(c, tcx, 1),
                                 emit_qk(c, tcx, 2)]
                    pending = None
                    # deferred outlin blocks of the previous pair-3 window,
                    # spread across this window's st iterations
                    outs = []
                    if tail is not None and tail[0] == H // 2 - 1:
                        outs = list(range(4 * tail[1], 4 * tail[1] + 4))
                    tail = None
                    # software pipeline: QK(st) is emitted before attV(st-1)
                    # so the PE never stalls behind the exp; the first two
                    # QKs of the NEXT window are pre-issued before this
                    # window's norm tail for the same reason.
                    for st in range(NS):
                        p_ps = (carry[st] if st < 3
                                else emit_qk(c, tcx, st))
                        if st in SCHRAUD_ST:
                            # exp via fp16 bit-trick on DVE: the int16
                            # y = a*score + b IS the fp16 encoding of
                            # 2^(log2e*score/8 - M) up to the (1+f)~2^f
                            # linear-mantissa approximation; max(0) clamps
                            # the below-cutoff tail that would bitcast to
                            # negative weights.
                            yi = schp.tile([128, 1024], I16, tag="pay")
                            nc.vector.tensor_scalar(
                                yi[:], p_ps[:], sch_a, sch_b,
                                ALU.mult, ALU.add)
                            pu = schp.tile([128, 1024], I16, tag="pac")
                            nc.vector.tensor_scalar_max(pu[:], yi[:], 0)
                            pa = pu.bitcast(FP)
                        else:
                            pa = pap.tile([128, 1024], FP, tag="pa")
                            nc.scalar.activation(pa[:], p_ps[:], AF.Exp,
                                                 scale=inv_sqrt_dk,
                                                 bias=ebias[:])
                        if st in (3, 6, 9, 12) and outs:
                            outlin_block(outs.pop(0))
                        if pending is not None:
                            attv_block(xA, xB, pending[0], pair,
                                       pending[1])
                        pending = (pa, st)
                    if w + 1 < len(windows):
                        npair, ntcx = windows[w + 1]
                        carry = [emit_qk(npair, ntcx, 0),
                                 emit_qk(npair, ntcx, 1)]
                    attv_block(xA, xB, pending[0], pair, pending[1])
                    if w + 1 < len(windows):
                        carry.append(emit_qk(npair, ntcx, 2))
                    # normalization: rows 0:64 = X^T_h, 64:128 = replicated
                    # denominators (kept at the window boundary -- the next
                    # window's first attV overwrites xA/xB and must trail
                    # these reads)
                    tsl = slice(tcx * 512, (tcx + 1) * 512)
                    rxA = rxp.tile([64, 512], F32, tag="rxA")
                    nc.vector.reciprocal(rxA[:], xA[64:128, :])
                    nc.vector.tensor_mul(XhatT[c][0:64, tsl],
                                         xA[0:64, :], rxA[:])
                    rxB = rxp.tile([64, 512], F32, tag="rxB")
                    nc.vector.reciprocal(rxB[:], xB[64:128, :])
                    nc.vector.tensor_mul(XhatT[c][64:128, tsl],
                                         xB[0:64, :], rxB[:])
                    # sweep-2 kagg quarters AFTER the norm ops so they never
                    # delay the next window's attV start on DVE
                    for _ in range(2):
                        if s2i < len(sweep2):
                            kagg_quarter(*sweep2[s2i])
                            s2i += 1
                    tail = (pair, tcx)

                # final window's outlin blocks
                for tb in range(4 * tail[1], 4 * tail[1] + 4):
                    outlin_block(tb)
                # drain any sweep-2 stragglers (shouldn't happen)
                while s2i < len(sweep2):
                    kagg_quarter(*sweep2[s2i])
                    s2i += 1


def _get_program():
    global _PROGRAM
    if _PROGRAM is None:
        _PROGRAM = _build_core_program()
    return _PROGRAM


def _prep_core_inputs(q, k, v, W0, b0, Wout, bout):
    """Host-side layout prep for one batch element (layout/dtype only)."""
    qTp = np.zeros((D, SQ), FP16)
    qTp[:, 0:S] = np.ascontiguousarray(q.T).astype(FP16)
    kTp = np.zeros((D, SQ), FP16)
    kTp[:, 0:S] = np.ascontiguousarray(k.T).astype(FP16)
    vTp = np.zeros((D, SPP), FP16)
    vTp[:, 0:S - 4] = np.ascontiguousarray(v[4:].T).astype(FP16)
    return {
        "qT": qTp,
        "kT": kTp,
        "vT": vTp,
        "W0": W0.astype(FP16),
        "Wout": Wout.astype(FP16),
        "b0": b0.reshape(1, D).astype(FP16),
        "bout": bout.reshape(1, D).astype(FP16),
        "zpad": np.zeros((4, H * VW), FP16),
        "ident": np.eye(128, dtype=np.float32).astype(FP16),
    }


def kernel(query, key, value, W0, b0, Wout, bout):
    query = np.asarray(query, np.float32)
    key = np.asarray(key, np.float32)
    value = np.asarray(value, np.float32)
    W0 = np.asarray(W0, np.float32)
    b0 = np.asarray(b0, np.float32)
    Wout = np.asarray(Wout, np.float32)
    bout = np.asarray(bout, np.float32)

    nc = _get_program()
    in_maps = [
        _prep_core_inputs(query[b], key[b], value[b], W0, b0, Wout, bout)
        for b in range(B)
    ]
    res = run_bass_kernel_spmd(nc, in_maps, list(range(N_CORES)))
    return np.stack([res.results[b]["out"] for b in range(B)], axis=0)
